# revision 13
# baseline (speedup 1.0000x reference)
"""Multi-head attention (B=2, T=2048, C=2048, H=16, causal, interleaved RoPE)
as a Bass/Tile kernel on 8 Trainium2 NeuronCores.

Sharding: core c handles batch b = c // 4 and heads 4*(c % 4) .. 4*(c % 4)+4.
Each core computes QKV for its heads, RoPE, causal attention, and the partial
output projection (row-parallel W_proj). Host sums the 4 partials per batch
and adds b_proj.

Device layouts (per core):
  - q, k are produced transposed [D=128(part), T] straight out of the QKV
    matmul (lhsT = W block, rhs = x^T).  The head dim is host-permuted to
    [even dims; odd dims] so interleaved RoPE is pure within-half DVE math.
  - v is produced natural [T(part), D] (lhsT = x^T block, rhs = W_v).
  - scores are computed transposed [Tk(part), Tq] so exp(scores)^T directly
    feeds the PV matmul as the moving operand; row sums come from a
    ones-vector matmul; 1/l normalization is applied at attention eviction.
Dtypes: QKV matmul in bf16; attention + projection matmuls in float32r
(TF32-like, 1 cycle/row); all accumulation fp32.
"""

import math

import numpy as np

P = 128  # partitions
B, T, C, H = 2, 2048, 2048, 16
D = C // H  # 128
NCORES = 8
GROUPS = 4  # head-groups per batch
HPC = H // GROUPS  # heads per core = 4
ROPE_BASE = 10000.0
NEG = -1e9

_CACHE = {}


def _dt():
    import concourse.mybir as mybir

    return mybir.dt


def build_nc(T=T, C=C, HPC=HPC, TCX=512, TC=512):
    """Build + compile the per-core Bass program (SPMD: same NEFF, 8 cores)."""
    import concourse.bacc as bacc
    import concourse.mybir as mybir
    import concourse.tile as tile

    dt = mybir.dt
    Act = mybir.ActivationFunctionType
    CS = C // P  # contraction slabs
    TB = T // P  # token blocks
    VC = HPC * D  # v columns per core (= 512 at full size)
    KBC = TC // P  # k-blocks per Tq chunk
    scale = 1.0 / math.sqrt(D)

    nc = bacc.Bacc("TRN2", target_bir_lowering=False, debug=False)
    with tile.TileContext(nc) as tc:
        with tc.tile_pool(name="dram", bufs=1, space="DRAM") as dram:

            def din(name, shape, dtype):
                return dram.tile(
                    shape, dtype, kind="ExternalInput", name=name, uniquify=False
                )

            xT = din("xT", [C, T], dt.bfloat16)  # x[b].T
            Wqkv = din("Wqkv", [C, 3 * VC], dt.bfloat16)  # [q|k|v] cols, q/k permuted
            bqk = din("bqk", [P, 2 * HPC], dt.float32)  # per-dim bias cols (q,k)
            bv = din("bv", [1, VC], dt.float32r)
            trigA = din("trigA", [P, T], dt.float32)  # [cos; cos]
            trigB = din("trigB", [P, T], dt.float32)  # [-sin; sin]
            mask4 = din("mask4", [P, KBC, TC], dt.float32)  # causal diag masks
            Wp = din("Wp", [VC, C], dt.float32r)  # W_proj rows for this core
            onesc = din("onesc", [P, 1], dt.float32r)
            onesr = din("onesr", [1, P], dt.float32r)
            out = dram.tile(
                [T, C], dt.float32, kind="ExternalOutput", name="out", uniquify=False
            )

            xT_r = xT.rearrange("(s p) t -> p s t", p=P)
            Wqkv_r = Wqkv.rearrange("(s p) n -> p s n", p=P)
            Wp_r = Wp.rearrange("(s p) n -> p s n", p=P)

            # ---- persistent across phases 1-2 ----
            with tc.tile_pool(name="persist", bufs=1) as persist:
                qrot = persist.tile([P, HPC, T], dt.bfloat16)
                krot = persist.tile([P, HPC, T], dt.bfloat16)
                v_sb = persist.tile([P, TB, VC], dt.float32r)
                ones_col = persist.tile([P, 1], dt.float32r)
                ones_row = persist.tile([1, P], dt.float32r)
                nc.sync.dma_start(out=ones_col, in_=onesc[:])
                nc.sync.dma_start(out=ones_row, in_=onesr[:])

                # ---------------- Phase 1: QKV + RoPE ----------------
                with tc.tile_pool(name="p1w", bufs=1) as p1w, tc.tile_pool(
                    name="p1xt", bufs=2
                ) as p1xt, tc.tile_pool(name="p1st", bufs=6) as p1st, tc.tile_pool(
                    name="p1ps", bufs=6, space="PSUM"
                ) as p1ps:
                    w_sb = p1w.tile([P, CS, 3 * VC], dt.bfloat16)
                    nc.sync.dma_start(out=w_sb, in_=Wqkv_r)
                    trigA_sb = p1w.tile([P, T], dt.float32)
                    trigB_sb = p1w.tile([P, T], dt.float32)
                    nc.sync.dma_start(out=trigA_sb, in_=trigA[:])
                    nc.sync.dma_start(out=trigB_sb, in_=trigB[:])
                    bqk_sb = p1w.tile([P, 2 * HPC], dt.float32)
                    nc.sync.dma_start(out=bqk_sb, in_=bqk[:])
                    bv_sb = p1w.tile([1, VC], dt.float32r)
                    nc.sync.dma_start(out=bv_sb, in_=bv[:])

                    for tx in range(T // TCX):
                        tsl = slice(tx * TCX, (tx + 1) * TCX)
                        xt_sb = p1xt.tile([P, CS, TCX], dt.bfloat16)
                        nc.sync.dma_start(out=xt_sb, in_=xT_r[:, :, tsl])
                        # q, k transposed per head: [D, TCX]
                        for qk in range(2):
                            dest = qrot if qk == 0 else krot
                            for h in range(HPC):
                                col = (qk * HPC + h) * D
                                ps = p1ps.tile([P, TCX], dt.float32)
                                for s in range(CS):
                                    nc.tensor.matmul(
                                        ps[:],
                                        w_sb[:, s, col : col + D],
                                        xt_sb[:, s, :],
                                        start=(s == 0),
                                        stop=(s == CS - 1),
                                    )
                                st = p1st.tile([P, TCX], dt.float32)
                                idx = qk * HPC + h
                                nc.scalar.activation(
                                    st[:],
                                    ps[:],
                                    Act.Identity,
                                    bias=bqk_sb[:, idx : idx + 1],
                                )
                                # RoPE: halves are [even dims; odd dims].
                                # rot = st*[cos;cos] + swap(st)*[-sin;sin]
                                HD = D // 2
                                sw = p1st.tile([P, TCX], dt.float32)
                                nc.sync.dma_start(
                                    out=sw[0:HD], in_=st[HD : 2 * HD]
                                )
                                nc.sync.dma_start(
                                    out=sw[HD : 2 * HD], in_=st[0:HD]
                                )
                                t1 = p1st.tile([P, TCX], dt.float32)
                                nc.vector.tensor_mul(t1[:], st[:], trigA_sb[:, tsl])
                                nc.vector.tensor_mul(sw[:], sw[:], trigB_sb[:, tsl])
                                nc.vector.tensor_add(dest[:, h, tsl], t1[:], sw[:])
                        # v natural rows
                        for tb in range(TCX // P):
                            kb = tx * (TCX // P) + tb
                            ps = p1ps.tile([P, VC], dt.float32)
                            for s in range(CS):
                                nc.tensor.matmul(
                                    ps[:],
                                    xt_sb[:, s, tb * P : (tb + 1) * P],
                                    w_sb[:, s, 2 * VC : 3 * VC],
                                    start=(s == 0),
                                    stop=False,
                                )
                            nc.tensor.matmul(
                                ps[:], ones_row[:], bv_sb[:], start=False, stop=True
                            )
                            nc.vector.tensor_copy(out=v_sb[:, kb, :], in_=ps[:])

                # ---------------- Phase 2: causal attention ----------------
                with tc.tile_pool(name="p2a", bufs=1) as p2a:
                    attnT = p2a.tile([P, HPC, T], dt.float32r)
                    wp_sb = p2a.tile([P, HPC, C], dt.float32r)
                    nc.sync.dma_start(out=wp_sb, in_=Wp_r)
                    mask_sb = p2a.tile([P, KBC, TC], dt.float32)
                    nc.sync.dma_start(out=mask_sb, in_=mask4[:])

                    with tc.tile_pool(name="p2probs", bufs=4) as p2probs, tc.tile_pool(
                        name="p2misc", bufs=4
                    ) as p2misc, tc.tile_pool(
                        name="p2sc", bufs=2, space="PSUM"
                    ) as p2sc, tc.tile_pool(
                        name="p2bc", bufs=1, space="PSUM"
                    ) as p2bc, tc.tile_pool(
                        name="p2acc", bufs=2, space="PSUM"
                    ) as p2acc, tc.tile_pool(
                        name="p2l", bufs=2, space="PSUM"
                    ) as p2l:
                        for h, tq in [
                            (h, tq) for h in range(HPC) for tq in range(T // TC)
                        ]:
                            qsl = slice(tq * TC, (tq + 1) * TC)
                            nkb = (tq + 1) * KBC
                            a_ps = p2acc.tile([P, TC], dt.float32)
                            l_ps = p2l.tile([1, TC], dt.float32)
                            for kb in range(nkb):
                                s_ps = p2sc.tile([P, TC], dt.float32)
                                nc.tensor.matmul(
                                    s_ps[:],
                                    krot[:, h, kb * P : (kb + 1) * P],
                                    qrot[:, h, qsl],
                                    start=True,
                                    stop=True,
                                )
                                j = kb - tq * KBC
                                if j >= 0:
                                    nc.vector.tensor_add(
                                        s_ps[:], s_ps[:], mask_sb[:, j, :]
                                    )
                                pt = p2probs.tile([P, TC], dt.float32r)
                                nc.scalar.activation(
                                    pt[:], s_ps[:], Act.Exp, scale=scale
                                )
                                nc.tensor.matmul(
                                    l_ps[:],
                                    ones_col[:],
                                    pt[:],
                                    start=(kb == 0),
                                    stop=(kb == nkb - 1),
                                )
                                nc.tensor.matmul(
                                    a_ps[:],
                                    v_sb[:, kb, h * D : (h + 1) * D],
                                    pt[:],
                                    start=(kb == 0),
                                    stop=(kb == nkb - 1),
                                )
                            # normalize by 1/l (broadcast over partitions via PE)
                            l_sb = p2misc.tile([1, TC], dt.float32r)
                            with nc.allow_low_precision("float32r is full width"):
                                nc.vector.reciprocal(l_sb[:], l_ps[:])
                            b_ps = p2bc.tile([P, TC], dt.float32)
                            nc.tensor.matmul(
                                b_ps[:], ones_row[:], l_sb[:], start=True, stop=True
                            )
                            b_sb = p2misc.tile([P, TC], dt.float32)
                            nc.scalar.copy(b_sb[:], b_ps[:])
                            nc.vector.tensor_mul(
                                attnT[:, h, qsl], a_ps[:], b_sb[:]
                            )

                    # ---------------- Phase 3: output projection ----------------
                    with tc.tile_pool(name="p3o", bufs=4) as p3o, tc.tile_pool(
                        name="p3ps", bufs=6, space="PSUM"
                    ) as p3ps:
                        NCH = 512
                        for tb in range(TB):
                            for ncol in range(C // NCH):
                                csl = slice(ncol * NCH, (ncol + 1) * NCH)
                                ps = p3ps.tile([P, NCH], dt.float32)
                                for j in range(HPC):
                                    nc.tensor.matmul(
                                        ps[:],
                                        attnT[:, j, tb * P : (tb + 1) * P],
                                        wp_sb[:, j, csl],
                                        start=(j == 0),
                                        stop=(j == HPC - 1),
                                    )
                                o_sb = p3o.tile([P, NCH], dt.float32)
                                nc.scalar.copy(o_sb[:], ps[:])
                                nc.sync.dma_start(
                                    out=out[tb * P : (tb + 1) * P, csl], in_=o_sb[:]
                                )
    nc.compile()
    return nc


# ---------------------------------------------------------------------------
# Host-side input prep
# ---------------------------------------------------------------------------


def _perm():
    """Head-dim permutation: interleaved (even,odd) -> [evens; odds]."""
    return np.concatenate([np.arange(0, D, 2), np.arange(1, D, 2)])


def prep_core_inputs(x_b, W_attn, b_attn, W_proj, heads, T=T, C=C, TC=512):
    """Build the per-core input map (numpy) for one (batch, head-group)."""
    import ml_dtypes

    bf16 = ml_dtypes.bfloat16
    perm = _perm()
    HPCl = len(heads)
    VC = HPCl * D
    KBC = TC // P

    Wq = W_attn[:, 0:C].reshape(C, H, D)
    Wk = W_attn[:, C : 2 * C].reshape(C, H, D)
    Wv = W_attn[:, 2 * C : 3 * C].reshape(C, H, D)
    bq = b_attn[0:C].reshape(H, D)
    bk = b_attn[C : 2 * C].reshape(H, D)
    bv = b_attn[2 * C : 3 * C].reshape(H, D)

    Wq_c = np.concatenate([Wq[:, h][:, perm] for h in heads], axis=1)  # [C, VC]
    Wk_c = np.concatenate([Wk[:, h][:, perm] for h in heads], axis=1)
    Wv_c = np.concatenate([Wv[:, h] for h in heads], axis=1)
    Wqkv = np.concatenate([Wq_c, Wk_c, Wv_c], axis=1).astype(bf16)  # [C, 3VC]

    bqk = np.stack(
        [bq[h][perm] for h in heads] + [bk[h][perm] for h in heads], axis=1
    ).astype(np.float32)  # [128, 2*HPC]
    bv_c = np.concatenate([bv[h] for h in heads]).reshape(1, VC).astype(np.float32)

    inv = ROPE_BASE ** (-np.arange(0, D, 2) / D)  # [64]
    ang = np.arange(T)[None, :] * inv[:, None]  # [64, T]
    cos, sin = np.cos(ang).astype(np.float32), np.sin(ang).astype(np.float32)
    trigA = np.concatenate([cos, cos], axis=0)  # [128, T]
    trigB = np.concatenate([-sin, sin], axis=0)

    # mask4[p, j, f] = 0 if (p + 128*j) <= f else NEG
    pp = np.arange(P)[:, None, None]
    jj = np.arange(KBC)[None, :, None]
    ff = np.arange(TC)[None, None, :]
    mask4 = np.where(pp + P * jj <= ff, 0.0, NEG).astype(np.float32)

    Wp_rows = np.concatenate(
        [W_proj[h * D : (h + 1) * D] for h in heads], axis=0
    ).astype(np.float32)  # [VC, C]

    return {
        "xT": np.ascontiguousarray(x_b.T).astype(bf16),
        "Wqkv": np.ascontiguousarray(Wqkv),
        "bqk": np.ascontiguousarray(bqk),
        "bv": bv_c,
        "trigA": trigA,
        "trigB": trigB,
        "mask4": np.ascontiguousarray(mask4),
        "Wp": np.ascontiguousarray(Wp_rows),
        "onesc": np.ones((P, 1), dtype=np.float32),
        "onesr": np.ones((1, P), dtype=np.float32),
    }


def make_in_maps(x, W_attn, b_attn, W_proj):
    in_maps = []
    for c in range(NCORES):
        b = c // GROUPS
        g = c % GROUPS
        heads = list(range(g * HPC, (g + 1) * HPC))
        in_maps.append(prep_core_inputs(x[b], W_attn, b_attn, W_proj, heads))
    return in_maps


def kernel(x, W_attn, b_attn, W_proj, b_proj):
    from concourse.bass_utils import run_bass_kernel_spmd

    if "nc" not in _CACHE:
        _CACHE["nc"] = build_nc()
    nc = _CACHE["nc"]

    x = np.asarray(x, dtype=np.float32)
    W_attn = np.asarray(W_attn, dtype=np.float32)
    b_attn = np.asarray(b_attn, dtype=np.float32)
    W_proj = np.asarray(W_proj, dtype=np.float32)
    b_proj = np.asarray(b_proj, dtype=np.float32)

    in_maps = make_in_maps(x, W_attn, b_attn, W_proj)
    res = run_bass_kernel_spmd(nc, in_maps, list(range(NCORES)))

    out = np.empty((B, T, C), dtype=np.float32)
    for b in range(B):
        acc = res.results[b * GROUPS]["out"].astype(np.float32).copy()
        for g in range(1, GROUPS):
            acc += res.results[b * GROUPS + g]["out"]
        out[b] = acc + b_proj[None, :]
    return out
